# revision 13
# baseline (speedup 1.0000x reference)
"""CTC loss (keras ctc_batch_cost semantics) on 8 Trainium2 NeuronCores.

Parity-normalized scan formulation. The CTC extended-state DP alternates
blank (even) and label (odd) states. All even states emit the same blank
probability pB_t, so normalizing the whole state vector by the running
blank product turns every even-state update into a pure shift-add and
every odd-state update into an affine recurrence in the emission RATIO
q_t = pl_t/pB_t. Each state-pair then reduces to three DVE instructions
over the full 256-step time axis:

    scanE:  E_t = delta_t * (E_{t-1} + O[j-1]_{t-1})     (cumsum-scan)
    stt:    b_t = K'_j * O[j-1]_{t-1} + E[j]_{t-1}       (fused mul-add)
    scanO:  O_t = qt[j]_t * (O_{t-1} + b_t)              (affine scan)

where qt = q * delta folds a per-(example,t) damping series delta chosen
from a mean-field surrogate so stored magnitudes stay O(1), and a
per-example tilt r applied every TILT_EVERY pairs (one tensor_scalar plus
host-folded K' = K*r) flattens the exponential state profile so bf16
storage holds the junction products. Forward (t<256) and reverse
(t>=256, states reversed) chains for 64 examples pack the 128 partitions
of each core; the final columns are gathered with two strided copies and
combined on the host in f64 with exact log-corrections for the blank
product, damping, and tilt ledgers.
"""
import numpy as np
import ml_dtypes

import concourse.bass as bass
import concourse.bacc as bacc
import concourse.mybir as mybir
from concourse import tile
from concourse.bass_utils import run_bass_kernel_spmd

B, T, C, L = 512, 512, 128, 64
S = 2 * L + 1
NCORES = 8
BS = B // NCORES        # 64 examples per core
HT = T // 2             # 256 timesteps per chain
BLANK = C - 1
EPS = 1e-7
TILT_EVERY = 8
PW = HT + 1             # per-pair series stride (1 guard col + 256)
QCHUNKS = (2, 6, 8, 8, 8, 8, 8, 8, 8)   # QS DMA chunk sizes in pairs
CF0, CF1 = 1.5689666, 2.17334313   # junction profile slope vs mean ln q

F32 = mybir.dt.float32
BF16 = mybir.dt.bfloat16
ADD = mybir.AluOpType.add
MULT = mybir.AluOpType.mult
bf16 = ml_dtypes.bfloat16

_CACHE = {}


HDR = HT + L            # header cols: dl series + kc columns


def _build_program():
    nc = bacc.Bacc("TRN2", target_bir_lowering=False, debug=False)
    qs = nc.dram_tensor("qs", [128, HDR + L * HT], BF16, kind="ExternalInput")
    gr = nc.dram_tensor("gr", [128, 1], F32, kind="ExternalInput")
    afin = nc.dram_tensor("afin", [128, 132], BF16, kind="ExternalOutput")

    with tile.TileContext(nc) as tc:
        with tc.tile_pool(name="static", bufs=1) as sp:
            ES = sp.tile([128, (L + 1) * PW], BF16)
            OS = sp.tile([128, L * PW], BF16)
            QS = sp.tile([128, HDR + L * HT], BF16)
            ZT = sp.tile([128, HT], BF16)
            GR = sp.tile([128, 1], F32)
            GT = [sp.tile([128, HT], BF16, name=f"gt{i}") for i in range(2)]
            BT = [sp.tile([128, HT], BF16, name=f"bt{i}") for i in range(2)]
            EX = sp.tile([128, 132], BF16)
            # header + first pairs land in chunk 0; gr is not needed until
            # the first tilt hop (pair 8), so its DMA rides later
            pos = 0
            for i, npair in enumerate(QCHUNKS):
                a = 0 if i == 0 else HDR + pos * HT
                b = HDR + (pos + npair) * HT
                nc.sync.dma_start(QS[:, a:b], qs[:, a:b])
                pos += npair
                if i == 1:
                    nc.sync.dma_start(GR[:, :], gr[:, :])
            nc.vector.memset(ZT[:, :], 0.0)
            # guard columns: E_{-1}[j] = [j == 0], O_{-1}[j] = 0
            nc.vector.memset(ES[:, 0:(L + 1) * PW:PW], 0.0)
            nc.vector.memset(OS[:, 0:L * PW:PW], 0.0)
            nc.vector.memset(ES[:, 0:1], 1.0)
            nc.vector.memset(EX[:, 129:132], 0.0)

            for j in range(L + 1):
                ob = (j - 1) * PW
                osh = ZT[:, 0:HT] if j == 0 else OS[:, ob:ob + HT]
                tilted = (j > 0) and (j % TILT_EVERY == 0)
                if tilted:
                    g = GT[(j // TILT_EVERY) % 2]
                    nc.vector.tensor_scalar_mul(g[:, :], osh, GR[:, 0:1])
                    d0e = g[:, 0:HT]
                else:
                    d0e = osh
                eb = j * PW
                nc.vector.tensor_tensor_scan(
                    ES[:, eb + 1:eb + 1 + HT], d0e, QS[:, 0:HT],
                    1.0 if j == 0 else 0.0, ADD, MULT)
                if j == L:
                    break
                b = BT[j % 2]
                nc.vector.scalar_tensor_tensor(
                    b[:, :], osh, QS[:, HT + j:HT + j + 1], ES[:, eb:eb + HT],
                    MULT, ADD)
                nc.vector.tensor_tensor_scan(
                    OS[:, ob + PW + 1:ob + PW + 1 + HT], b[:, 0:HT],
                    QS[:, HDR + j * HT:HDR + (j + 1) * HT], 0.0, ADD, MULT)
            nc.vector.tensor_copy(EX[:, 0:L + 1], ES[:, HT::PW])
            nc.vector.tensor_copy(EX[:, L + 1:S], OS[:, HT::PW])
            nc.sync.dma_start(afin[:, :], EX[:, :])
    nc.compile()
    return nc


def _host_prep(y_true, y_pred):
    yt = np.asarray(y_true)
    yp = np.asarray(y_pred, dtype=np.float32)
    pB = yp[:, :, BLANK].astype(np.float64) + EPS            # [B, T]
    pl = (np.take_along_axis(yp, yt[:, None, :].astype(np.int64), axis=2)
          .astype(np.float64) + EPS)                          # [B, T, L]

    # fwd chain (t < HT) and bwd chain (reversed time + labels)
    q_f = pl[:, :HT, :] / pB[:, :HT, None]
    q_b = pl[:, :HT - 1:-1, ::-1] / pB[:, :HT - 1:-1, None]
    K_f = np.zeros((B, L))
    K_f[:, 1:] = (yt[:, 1:] != yt[:, :-1]).astype(np.float64)
    K_b = np.zeros((B, L))
    K_b[:, 1:] = (yt[:, ::-1][:, 1:] != yt[:, ::-1][:, :-1]).astype(np.float64)

    def chain_params(q):
        lnq = np.log(q).mean(axis=(1, 2))
        slope = CF0 * lnq + CF1
        gam = np.exp(-slope)
        r = gam ** (2 * TILT_EVERY)
        # damping from tilted 2-state mean-field surrogate
        e = np.ones((B,)); o = np.zeros((B,))
        g = np.empty((B, HT))
        qb = q.mean(axis=2)
        for t in range(HT):
            e2 = e + gam * o
            o2 = qb[:, t] * (o + gam * e + gam * gam * o)
            z2 = e2 + o2
            g[:, t] = z2 / (e + o)
            e, o = e2 / z2, o2 / z2
        delta = 1.0 / g
        return r, delta

    r_f, d_f = chain_params(q_f)
    r_b, d_b = chain_params(q_b)

    def pack(q, K, r, delta):
        # qs rows: [dl series | K' columns | pair-major q*delta series]
        n = q.shape[0]
        qt = (q * delta[:, :, None]).transpose(0, 2, 1)       # [n, L, HT]
        kc = K.copy()
        for j in range(TILT_EVERY, L, TILT_EVERY):
            kc[:, j] *= r
        qs = np.concatenate(
            [delta, kc, qt.reshape(n, L * HT)], axis=1).astype(bf16)
        return qs, r.astype(np.float32)

    qs_f, gr_f = pack(q_f, K_f, r_f, d_f)
    qs_b, gr_b = pack(q_b, K_b, r_b, d_b)

    in_maps = []
    for ci in range(NCORES):
        ex = slice(ci * BS, (ci + 1) * BS)
        in_maps.append({
            "qs": np.concatenate([qs_f[ex], qs_b[ex]], axis=0),
            "gr": np.concatenate([gr_f[ex], gr_b[ex]], axis=0)[:, None],
        })
    aux = (pB, r_f, r_b, d_f, d_b, yt)
    return in_maps, aux


def _host_combine(afin, aux):
    pB, r_f, r_b, d_f, d_b, yt = aux
    af_s = afin[:, :BS, :].reshape(B, 132).astype(np.float64)
    ab_s = afin[:, BS:, :].reshape(B, 132).astype(np.float64)

    # un-tilt ledger: pair j carries floor(j / TILT_EVERY) factors of r
    nt = np.floor_divide(np.arange(L + 1), TILT_EVERY)
    af = np.zeros((B, S)); ab = np.zeros((B, S))
    af[:, 0::2] = af_s[:, 0:L + 1] * r_f[:, None] ** (-nt[None, :])
    af[:, 1::2] = af_s[:, L + 1:S] * r_f[:, None] ** (-nt[None, :L])
    ab[:, 0::2] = ab_s[:, 0:L + 1] * r_b[:, None] ** (-nt[None, :])
    ab[:, 1::2] = ab_s[:, L + 1:S] * r_b[:, None] ** (-nt[None, :L])

    ext = np.full((B, S), BLANK, np.int64)
    ext[:, 1::2] = yt
    cs = np.zeros((B, S))
    cs[:, 2:] = ((ext[:, 2:] != BLANK)
                 & (ext[:, 2:] != ext[:, :-2])).astype(np.float64)
    zg = np.zeros((B, S + 2))
    zg[:, 2:] = af
    z = zg[:, 2:] + zg[:, 1:-1] + cs * zg[:, 0:-2]
    dot = (z * ab[:, ::-1]).sum(axis=1)

    lnF = np.log(pB).sum(axis=1)
    lnD = np.log(d_f).sum(axis=1) + np.log(d_b).sum(axis=1)
    ll = np.log(np.maximum(dot, 1e-300)) + lnF - lnD
    return (-ll[:, None]).astype(np.float32)


def kernel(y_true, y_pred):
    in_maps, aux = _host_prep(y_true, y_pred)
    if "nc" not in _CACHE:
        _CACHE["nc"] = _build_program()
    nc = _CACHE["nc"]
    res = run_bass_kernel_spmd(nc, in_maps, core_ids=list(range(NCORES)))
    afin = np.stack([np.asarray(res.results[i]["afin"], dtype=np.float32)
                     for i in range(NCORES)])
    return _host_combine(afin, aux)


# revision 14
# speedup vs baseline: 1.0100x; 1.0100x over previous
"""CTC loss (keras ctc_batch_cost semantics) on 8 Trainium2 NeuronCores.

Parity-normalized scan formulation. The CTC extended-state DP alternates
blank (even) and label (odd) states. All even states emit the same blank
probability pB_t, so normalizing the whole state vector by the running
blank product turns every even-state update into a pure shift-add and
every odd-state update into an affine recurrence in the emission RATIO
q_t = pl_t/pB_t. Each state-pair then reduces to three DVE instructions
over the full 256-step time axis:

    scanE:  E_t = delta_t * (E_{t-1} + O[j-1]_{t-1})     (cumsum-scan)
    stt:    b_t = K'_j * O[j-1]_{t-1} + E[j]_{t-1}       (fused mul-add)
    scanO:  O_t = qt[j]_t * (O_{t-1} + b_t)              (affine scan)

where qt = q * delta folds a per-(example,t) damping series delta chosen
from a mean-field surrogate so stored magnitudes stay O(1), and a
per-example tilt r applied every TILT_EVERY pairs (one tensor_scalar plus
host-folded K' = K*r) flattens the exponential state profile so bf16
storage holds the junction products. Forward (t<256) and reverse
(t>=256, states reversed) chains for 64 examples pack the 128 partitions
of each core; the final columns are gathered with two strided copies and
combined on the host in f64 with exact log-corrections for the blank
product, damping, and tilt ledgers.
"""
import numpy as np
import ml_dtypes

import concourse.bass as bass
import concourse.bacc as bacc
import concourse.mybir as mybir
from concourse import tile
from concourse.bass_utils import run_bass_kernel_spmd

B, T, C, L = 512, 512, 128, 64
S = 2 * L + 1
NCORES = 8
BS = B // NCORES        # 64 examples per core
HT = T // 2             # 256 timesteps per chain
BLANK = C - 1
EPS = 1e-7
TILT_EVERY = 16
PW = HT + 1             # per-pair series stride (1 guard col + 256)
QCHUNKS = (2, 6, 8, 8, 8, 8, 8, 8, 8)   # QS DMA chunk sizes in pairs
CF0, CF1 = 1.5689666, 2.17334313   # junction profile slope vs mean ln q

F32 = mybir.dt.float32
BF16 = mybir.dt.bfloat16
ADD = mybir.AluOpType.add
MULT = mybir.AluOpType.mult
bf16 = ml_dtypes.bfloat16

_CACHE = {}


HDR = HT + L            # header cols: dl series + kc columns


def _build_program():
    nc = bacc.Bacc("TRN2", target_bir_lowering=False, debug=False)
    qs = nc.dram_tensor("qs", [128, HDR + L * HT], BF16, kind="ExternalInput")
    gr = nc.dram_tensor("gr", [128, 1], F32, kind="ExternalInput")
    afin = nc.dram_tensor("afin", [128, 132], BF16, kind="ExternalOutput")

    with tile.TileContext(nc) as tc:
        with tc.tile_pool(name="static", bufs=1) as sp:
            ES = sp.tile([128, (L + 1) * PW], BF16)
            OS = sp.tile([128, L * PW], BF16)
            QS = sp.tile([128, HDR + L * HT], BF16)
            ZT = sp.tile([128, HT], BF16)
            GR = sp.tile([128, 1], F32)
            GT = [sp.tile([128, HT], BF16, name=f"gt{i}") for i in range(2)]
            BT = [sp.tile([128, HT], BF16, name=f"bt{i}") for i in range(2)]
            EX = sp.tile([128, 132], BF16)
            # header + first pairs land in chunk 0; gr is not needed until
            # the first tilt hop (pair 8), so its DMA rides later
            pos = 0
            for i, npair in enumerate(QCHUNKS):
                a = 0 if i == 0 else HDR + pos * HT
                b = HDR + (pos + npair) * HT
                nc.sync.dma_start(QS[:, a:b], qs[:, a:b])
                pos += npair
                if i == 1:
                    nc.sync.dma_start(GR[:, :], gr[:, :])
            nc.vector.memset(ZT[:, :], 0.0)
            # guard columns: E_{-1}[j] = [j == 0], O_{-1}[j] = 0
            nc.vector.memset(ES[:, 0:(L + 1) * PW:PW], 0.0)
            nc.vector.memset(OS[:, 0:L * PW:PW], 0.0)
            nc.vector.memset(ES[:, 0:1], 1.0)
            nc.vector.memset(EX[:, 129:132], 0.0)

            for j in range(L + 1):
                ob = (j - 1) * PW
                osh = ZT[:, 0:HT] if j == 0 else OS[:, ob:ob + HT]
                tilted = (j > 0) and (j % TILT_EVERY == 0)
                if tilted:
                    g = GT[(j // TILT_EVERY) % 2]
                    nc.vector.tensor_scalar_mul(g[:, :], osh, GR[:, 0:1])
                    d0e = g[:, 0:HT]
                else:
                    d0e = osh
                eb = j * PW
                nc.vector.tensor_tensor_scan(
                    ES[:, eb + 1:eb + 1 + HT], d0e, QS[:, 0:HT],
                    1.0 if j == 0 else 0.0, ADD, MULT)
                if j == L:
                    break
                b = BT[j % 2]
                nc.vector.scalar_tensor_tensor(
                    b[:, :], osh, QS[:, HT + j:HT + j + 1], ES[:, eb:eb + HT],
                    MULT, ADD)
                nc.vector.tensor_tensor_scan(
                    OS[:, ob + PW + 1:ob + PW + 1 + HT], b[:, 0:HT],
                    QS[:, HDR + j * HT:HDR + (j + 1) * HT], 0.0, ADD, MULT)
            nc.vector.tensor_copy(EX[:, 0:L + 1], ES[:, HT::PW])
            nc.vector.tensor_copy(EX[:, L + 1:S], OS[:, HT::PW])
            nc.sync.dma_start(afin[:, :], EX[:, :])
    nc.compile()
    return nc


def _host_prep(y_true, y_pred):
    yt = np.asarray(y_true)
    yp = np.asarray(y_pred, dtype=np.float32)
    pB = yp[:, :, BLANK].astype(np.float64) + EPS            # [B, T]
    pl = (np.take_along_axis(yp, yt[:, None, :].astype(np.int64), axis=2)
          .astype(np.float64) + EPS)                          # [B, T, L]

    # fwd chain (t < HT) and bwd chain (reversed time + labels)
    q_f = pl[:, :HT, :] / pB[:, :HT, None]
    q_b = pl[:, :HT - 1:-1, ::-1] / pB[:, :HT - 1:-1, None]
    K_f = np.zeros((B, L))
    K_f[:, 1:] = (yt[:, 1:] != yt[:, :-1]).astype(np.float64)
    K_b = np.zeros((B, L))
    K_b[:, 1:] = (yt[:, ::-1][:, 1:] != yt[:, ::-1][:, :-1]).astype(np.float64)

    def chain_params(q):
        lnq = np.log(q).mean(axis=(1, 2))
        slope = CF0 * lnq + CF1
        gam = np.exp(-slope)
        r = gam ** (2 * TILT_EVERY)
        # damping from tilted 2-state mean-field surrogate
        e = np.ones((B,)); o = np.zeros((B,))
        g = np.empty((B, HT))
        qb = q.mean(axis=2)
        for t in range(HT):
            e2 = e + gam * o
            o2 = qb[:, t] * (o + gam * e + gam * gam * o)
            z2 = e2 + o2
            g[:, t] = z2 / (e + o)
            e, o = e2 / z2, o2 / z2
        delta = np.exp(-22.0 / 256.0) / g
        return r, delta

    r_f, d_f = chain_params(q_f)
    r_b, d_b = chain_params(q_b)

    def pack(q, K, r, delta):
        # qs rows: [dl series | K' columns | pair-major q*delta series]
        n = q.shape[0]
        qt = (q * delta[:, :, None]).transpose(0, 2, 1)       # [n, L, HT]
        kc = K.copy()
        for j in range(TILT_EVERY, L, TILT_EVERY):
            kc[:, j] *= r
        qs = np.concatenate(
            [delta, kc, qt.reshape(n, L * HT)], axis=1).astype(bf16)
        return qs, r.astype(np.float32)

    qs_f, gr_f = pack(q_f, K_f, r_f, d_f)
    qs_b, gr_b = pack(q_b, K_b, r_b, d_b)

    in_maps = []
    for ci in range(NCORES):
        ex = slice(ci * BS, (ci + 1) * BS)
        in_maps.append({
            "qs": np.concatenate([qs_f[ex], qs_b[ex]], axis=0),
            "gr": np.concatenate([gr_f[ex], gr_b[ex]], axis=0)[:, None],
        })
    aux = (pB, r_f, r_b, d_f, d_b, yt)
    return in_maps, aux


def _host_combine(afin, aux):
    pB, r_f, r_b, d_f, d_b, yt = aux
    af_s = afin[:, :BS, :].reshape(B, 132).astype(np.float64)
    ab_s = afin[:, BS:, :].reshape(B, 132).astype(np.float64)

    # un-tilt ledger: pair j carries floor(j / TILT_EVERY) factors of r
    nt = np.floor_divide(np.arange(L + 1), TILT_EVERY)
    af = np.zeros((B, S)); ab = np.zeros((B, S))
    af[:, 0::2] = af_s[:, 0:L + 1] * r_f[:, None] ** (-nt[None, :])
    af[:, 1::2] = af_s[:, L + 1:S] * r_f[:, None] ** (-nt[None, :L])
    ab[:, 0::2] = ab_s[:, 0:L + 1] * r_b[:, None] ** (-nt[None, :])
    ab[:, 1::2] = ab_s[:, L + 1:S] * r_b[:, None] ** (-nt[None, :L])

    ext = np.full((B, S), BLANK, np.int64)
    ext[:, 1::2] = yt
    cs = np.zeros((B, S))
    cs[:, 2:] = ((ext[:, 2:] != BLANK)
                 & (ext[:, 2:] != ext[:, :-2])).astype(np.float64)
    zg = np.zeros((B, S + 2))
    zg[:, 2:] = af
    z = zg[:, 2:] + zg[:, 1:-1] + cs * zg[:, 0:-2]
    dot = (z * ab[:, ::-1]).sum(axis=1)

    lnF = np.log(pB).sum(axis=1)
    lnD = np.log(d_f).sum(axis=1) + np.log(d_b).sum(axis=1)
    ll = np.log(np.maximum(dot, 1e-300)) + lnF - lnD
    return (-ll[:, None]).astype(np.float32)


def kernel(y_true, y_pred):
    in_maps, aux = _host_prep(y_true, y_pred)
    if "nc" not in _CACHE:
        _CACHE["nc"] = _build_program()
    nc = _CACHE["nc"]
    res = run_bass_kernel_spmd(nc, in_maps, core_ids=list(range(NCORES)))
    afin = np.stack([np.asarray(res.results[i]["afin"], dtype=np.float32)
                     for i in range(NCORES)])
    return _host_combine(afin, aux)


# revision 16
# speedup vs baseline: 1.0176x; 1.0075x over previous
"""CTC loss (keras ctc_batch_cost semantics) on 8 Trainium2 NeuronCores.

Parity-normalized scan formulation. The CTC extended-state DP alternates
blank (even) and label (odd) states. All even states emit the same blank
probability pB_t, so normalizing the whole state vector by the running
blank product turns every even-state update into a pure shift-add and
every odd-state update into an affine recurrence in the emission RATIO
q_t = pl_t/pB_t. Each state-pair then reduces to three DVE instructions
over the full 256-step time axis:

    scanE:  E_t = delta_t * (E_{t-1} + O[j-1]_{t-1})     (cumsum-scan)
    stt:    b_t = K'_j * O[j-1]_{t-1} + E[j]_{t-1}       (fused mul-add)
    scanO:  O_t = qt[j]_t * (O_{t-1} + b_t)              (affine scan)

where qt = q * delta folds a per-(example,t) damping series delta chosen
from a mean-field surrogate so stored magnitudes stay O(1), and a
per-example tilt r applied every TILT_EVERY pairs (one tensor_scalar plus
host-folded K' = K*r) flattens the exponential state profile so bf16
storage holds the junction products. Forward (t<256) and reverse
(t>=256, states reversed) chains for 64 examples pack the 128 partitions
of each core; the final columns are gathered with two strided copies and
combined on the host in f64 with exact log-corrections for the blank
product, damping, and tilt ledgers.
"""
import numpy as np
import ml_dtypes

import concourse.bass as bass
import concourse.bacc as bacc
import concourse.mybir as mybir
from concourse import tile
from concourse.bass_utils import run_bass_kernel_spmd

B, T, C, L = 512, 512, 128, 64
S = 2 * L + 1
NCORES = 8
BS = B // NCORES        # 64 examples per core
HT = T // 2             # 256 timesteps per chain
BLANK = C - 1
EPS = 1e-7
TILT_EVERY = 16
PW = HT + 1             # per-pair series stride (1 guard col + 256)
QCHUNKS = (2, 6, 8, 8, 8, 8, 8, 8, 8)   # QS DMA chunk sizes in pairs
CF0, CF1 = 1.5689666, 2.17334313   # junction profile slope vs mean ln q

F32 = mybir.dt.float32
BF16 = mybir.dt.bfloat16
ADD = mybir.AluOpType.add
MULT = mybir.AluOpType.mult
bf16 = ml_dtypes.bfloat16

_CACHE = {}


HDR = 2 * HT + L        # header cols: dl series + E[0] series + kc columns


def _build_program():
    nc = bacc.Bacc("TRN2", target_bir_lowering=False, debug=False)
    qs = nc.dram_tensor("qs", [128, HDR + L * HT], BF16, kind="ExternalInput")
    gr = nc.dram_tensor("gr", [128, 1], F32, kind="ExternalInput")
    afin = nc.dram_tensor("afin", [128, 132], BF16, kind="ExternalOutput")

    with tile.TileContext(nc) as tc:
        with tc.tile_pool(name="static", bufs=1) as sp:
            ES = sp.tile([128, (L + 1) * PW], BF16)
            OS = sp.tile([128, L * PW], BF16)
            QS = sp.tile([128, HDR + L * HT], BF16)
            GR = sp.tile([128, 1], F32)
            GT = [sp.tile([128, HT], BF16, name=f"gt{i}") for i in range(2)]
            BT = [sp.tile([128, HT], BF16, name=f"bt{i}") for i in range(2)]
            EX = sp.tile([128, 132], BF16)
            # header + first pairs land in chunk 0; gr is not needed until
            # the first tilt hop (pair 8), so its DMA rides later
            pos = 0
            for i, npair in enumerate(QCHUNKS):
                a = 0 if i == 0 else HDR + pos * HT
                b = HDR + (pos + npair) * HT
                nc.sync.dma_start(QS[:, a:b], qs[:, a:b])
                pos += npair
                if i == 1:
                    nc.sync.dma_start(GR[:, :], gr[:, :])
            # guard columns: E_{-1}[j] = 0 (j >= 1), O_{-1}[j] = 0
            nc.vector.memset(ES[:, 0:(L + 1) * PW:PW], 0.0)
            nc.vector.memset(OS[:, 0:L * PW:PW], 0.0)
            nc.vector.memset(EX[:, 129:132], 0.0)

            # pair 0: E[0] is the shipped delta-cumprod and K_0 = 0, so
            # b_0 is just its shift -- one scan total
            nc.vector.tensor_tensor_scan(
                OS[:, 1:1 + HT], QS[:, HT:2 * HT],
                QS[:, HDR:HDR + HT], 0.0, ADD, MULT)
            for j in range(1, L + 1):
                ob = (j - 1) * PW
                osh = OS[:, ob:ob + HT]
                tilted = (j % TILT_EVERY == 0)
                if tilted:
                    g = GT[(j // TILT_EVERY) % 2]
                    nc.vector.tensor_scalar_mul(g[:, :], osh, GR[:, 0:1])
                    d0e = g[:, 0:HT]
                else:
                    d0e = osh
                eb = j * PW
                nc.vector.tensor_tensor_scan(
                    ES[:, eb + 1:eb + 1 + HT], d0e, QS[:, 0:HT],
                    0.0, ADD, MULT)
                if j == L:
                    break
                b = BT[j % 2]
                nc.vector.scalar_tensor_tensor(
                    b[:, :], osh, QS[:, 2 * HT + j:2 * HT + j + 1],
                    ES[:, eb:eb + HT], MULT, ADD)
                nc.vector.tensor_tensor_scan(
                    OS[:, ob + PW + 1:ob + PW + 1 + HT], b[:, 0:HT],
                    QS[:, HDR + j * HT:HDR + (j + 1) * HT], 0.0, ADD, MULT)
            nc.vector.memset(EX[:, 0:1], 0.0)
            nc.vector.tensor_copy(EX[:, 1:L + 1], ES[:, PW + HT::PW])
            nc.vector.tensor_copy(EX[:, L + 1:S], OS[:, HT::PW])
            nc.sync.dma_start(afin[:, :], EX[:, :])
    nc.compile()
    return nc


def _host_prep(y_true, y_pred):
    yt = np.asarray(y_true)
    yp = np.asarray(y_pred, dtype=np.float32)
    pB = yp[:, :, BLANK].astype(np.float64) + EPS            # [B, T]
    pl = (np.take_along_axis(yp, yt[:, None, :].astype(np.int64), axis=2)
          .astype(np.float64) + EPS)                          # [B, T, L]

    # fwd chain (t < HT) and bwd chain (reversed time + labels)
    q_f = pl[:, :HT, :] / pB[:, :HT, None]
    q_b = pl[:, :HT - 1:-1, ::-1] / pB[:, :HT - 1:-1, None]
    K_f = np.zeros((B, L))
    K_f[:, 1:] = (yt[:, 1:] != yt[:, :-1]).astype(np.float64)
    K_b = np.zeros((B, L))
    K_b[:, 1:] = (yt[:, ::-1][:, 1:] != yt[:, ::-1][:, :-1]).astype(np.float64)

    def chain_params(q):
        lnq = np.log(q).mean(axis=(1, 2))
        slope = CF0 * lnq + CF1
        gam = np.exp(-slope)
        r = gam ** (2 * TILT_EVERY)
        # damping from tilted 2-state mean-field surrogate
        e = np.ones((B,)); o = np.zeros((B,))
        g = np.empty((B, HT))
        qb = q.mean(axis=2)
        for t in range(HT):
            e2 = e + gam * o
            o2 = qb[:, t] * (o + gam * e + gam * gam * o)
            z2 = e2 + o2
            g[:, t] = z2 / (e + o)
            e, o = e2 / z2, o2 / z2
        delta = np.exp(-22.0 / 256.0) / g
        return r, delta

    r_f, d_f = chain_params(q_f)
    r_b, d_b = chain_params(q_b)

    def pack(q, K, r, delta):
        # qs rows: [dl | E0 series (shifted cumprod) | K' | q*delta series]
        n = q.shape[0]
        qt = (q * delta[:, :, None]).transpose(0, 2, 1)       # [n, L, HT]
        kc = K.copy()
        for j in range(TILT_EVERY, L, TILT_EVERY):
            kc[:, j] *= r
        ecp = np.ones((n, HT))
        ecp[:, 1:] = np.cumprod(delta[:, :HT - 1], axis=1)
        qs = np.concatenate(
            [delta, ecp, kc, qt.reshape(n, L * HT)], axis=1).astype(bf16)
        return qs, r.astype(np.float32)

    qs_f, gr_f = pack(q_f, K_f, r_f, d_f)
    qs_b, gr_b = pack(q_b, K_b, r_b, d_b)

    in_maps = []
    for ci in range(NCORES):
        ex = slice(ci * BS, (ci + 1) * BS)
        in_maps.append({
            "qs": np.concatenate([qs_f[ex], qs_b[ex]], axis=0),
            "gr": np.concatenate([gr_f[ex], gr_b[ex]], axis=0)[:, None],
        })
    aux = (pB, r_f, r_b, d_f, d_b, yt)
    return in_maps, aux


def _host_combine(afin, aux):
    pB, r_f, r_b, d_f, d_b, yt = aux
    af_s = afin[:, :BS, :].reshape(B, 132).astype(np.float64)
    ab_s = afin[:, BS:, :].reshape(B, 132).astype(np.float64)
    af_s[:, 0] = np.exp(np.log(d_f).sum(axis=1))
    ab_s[:, 0] = np.exp(np.log(d_b).sum(axis=1))

    # un-tilt ledger: pair j carries floor(j / TILT_EVERY) factors of r
    nt = np.floor_divide(np.arange(L + 1), TILT_EVERY)
    af = np.zeros((B, S)); ab = np.zeros((B, S))
    af[:, 0::2] = af_s[:, 0:L + 1] * r_f[:, None] ** (-nt[None, :])
    af[:, 1::2] = af_s[:, L + 1:S] * r_f[:, None] ** (-nt[None, :L])
    ab[:, 0::2] = ab_s[:, 0:L + 1] * r_b[:, None] ** (-nt[None, :])
    ab[:, 1::2] = ab_s[:, L + 1:S] * r_b[:, None] ** (-nt[None, :L])

    ext = np.full((B, S), BLANK, np.int64)
    ext[:, 1::2] = yt
    cs = np.zeros((B, S))
    cs[:, 2:] = ((ext[:, 2:] != BLANK)
                 & (ext[:, 2:] != ext[:, :-2])).astype(np.float64)
    zg = np.zeros((B, S + 2))
    zg[:, 2:] = af
    z = zg[:, 2:] + zg[:, 1:-1] + cs * zg[:, 0:-2]
    dot = (z * ab[:, ::-1]).sum(axis=1)

    lnF = np.log(pB).sum(axis=1)
    lnD = np.log(d_f).sum(axis=1) + np.log(d_b).sum(axis=1)
    ll = np.log(np.maximum(dot, 1e-300)) + lnF - lnD
    return (-ll[:, None]).astype(np.float32)


def kernel(y_true, y_pred):
    in_maps, aux = _host_prep(y_true, y_pred)
    if "nc" not in _CACHE:
        _CACHE["nc"] = _build_program()
    nc = _CACHE["nc"]
    res = run_bass_kernel_spmd(nc, in_maps, core_ids=list(range(NCORES)))
    afin = np.stack([np.asarray(res.results[i]["afin"], dtype=np.float32)
                     for i in range(NCORES)])
    return _host_combine(afin, aux)


# revision 17
# speedup vs baseline: 1.0176x; 1.0000x over previous
"""CTC loss (keras ctc_batch_cost semantics) on 8 Trainium2 NeuronCores.

Parity-normalized scan formulation. The CTC extended-state DP alternates
blank (even) and label (odd) states. All even states emit the same blank
probability pB_t, so normalizing the whole state vector by the running
blank product turns every even-state update into a pure shift-add and
every odd-state update into an affine recurrence in the emission RATIO
q_t = pl_t/pB_t. Each state-pair then reduces to three DVE instructions
over the full 256-step time axis:

    scanE:  E_t = delta_t * (E_{t-1} + O[j-1]_{t-1})     (cumsum-scan)
    stt:    b_t = K'_j * O[j-1]_{t-1} + E[j]_{t-1}       (fused mul-add)
    scanO:  O_t = qt[j]_t * (O_{t-1} + b_t)              (affine scan)

where qt = q * delta folds a per-(example,t) damping series delta chosen
from a mean-field surrogate so stored magnitudes stay O(1), and a
per-example tilt r applied every TILT_EVERY pairs (one tensor_scalar plus
host-folded K' = K*r) flattens the exponential state profile so bf16
storage holds the junction products. Forward (t<256) and reverse
(t>=256, states reversed) chains for 64 examples pack the 128 partitions
of each core; the final columns are gathered with two strided copies and
combined on the host in f64 with exact log-corrections for the blank
product, damping, and tilt ledgers.
"""
import numpy as np
import ml_dtypes

import concourse.bass as bass
import concourse.bacc as bacc
import concourse.mybir as mybir
from concourse import tile
from concourse.bass_utils import run_bass_kernel_spmd

B, T, C, L = 512, 512, 128, 64
S = 2 * L + 1
NCORES = 8
BS = B // NCORES        # 64 examples per core
HT = T // 2             # 256 timesteps per chain
BLANK = C - 1
EPS = 1e-7
TILT_EVERY = 16
PW = HT + 1             # per-pair series stride (1 guard col + 256)
QCHUNKS = (2, 6, 8, 8, 8, 8, 8, 8, 8)   # QS DMA chunk sizes in pairs
CF0, CF1 = 1.5689666, 2.17334313   # junction profile slope vs mean ln q

F32 = mybir.dt.float32
BF16 = mybir.dt.bfloat16
ADD = mybir.AluOpType.add
MULT = mybir.AluOpType.mult
bf16 = ml_dtypes.bfloat16

_CACHE = {}


HDR = 2 * HT + L        # header cols: dl series + E[0] series + kc columns


def _build_program():
    nc = bacc.Bacc("TRN2", target_bir_lowering=False, debug=False)
    qs = nc.dram_tensor("qs", [128, HDR + L * HT], BF16, kind="ExternalInput")
    gr = nc.dram_tensor("gr", [128, 1], F32, kind="ExternalInput")
    afin = nc.dram_tensor("afin", [128, 132], BF16, kind="ExternalOutput")

    with tile.TileContext(nc) as tc:
        with tc.tile_pool(name="static", bufs=1) as sp:
            ES = sp.tile([128, (L + 1) * PW], BF16)
            OS = sp.tile([128, L * PW], BF16)
            QS = sp.tile([128, HDR + L * HT], BF16)
            GR = sp.tile([128, 1], F32)
            GT = [sp.tile([128, HT], BF16, name=f"gt{i}") for i in range(2)]
            BT = [sp.tile([128, HT], BF16, name=f"bt{i}") for i in range(2)]
            EX = sp.tile([128, 132], BF16)
            # header + first pairs land in chunk 0; gr is not needed until
            # the first tilt hop (pair 8), so its DMA rides later
            pos = 0
            for i, npair in enumerate(QCHUNKS):
                a = 0 if i == 0 else HDR + pos * HT
                b = HDR + (pos + npair) * HT
                nc.sync.dma_start(QS[:, a:b], qs[:, a:b])
                pos += npair
                if i == 1:
                    nc.sync.dma_start(GR[:, :], gr[:, :])
            # guard columns: E_{-1}[j] = 0 (j >= 1), O_{-1}[j] = 0
            nc.vector.memset(ES[:, 0:(L + 1) * PW:PW], 0.0)
            nc.vector.memset(OS[:, 0:L * PW:PW], 0.0)
            nc.vector.memset(EX[:, 129:132], 0.0)

            # pair 0: E[0] is the shipped delta-cumprod and K_0 = 0, so
            # b_0 is just its shift -- one scan total
            nc.vector.tensor_tensor_scan(
                OS[:, 1:1 + HT], QS[:, HT:2 * HT],
                QS[:, HDR:HDR + HT], 0.0, ADD, MULT)
            for j in range(1, L + 1):
                ob = (j - 1) * PW
                osh = OS[:, ob:ob + HT]
                tilted = (j % TILT_EVERY == 0)
                if tilted:
                    g = GT[(j // TILT_EVERY) % 2]
                    nc.vector.tensor_scalar_mul(g[:, :], osh, GR[:, 0:1])
                    d0e = g[:, 0:HT]
                else:
                    d0e = osh
                eb = j * PW
                nc.vector.tensor_tensor_scan(
                    ES[:, eb + 1:eb + 1 + HT], d0e, QS[:, 0:HT],
                    0.0, ADD, MULT)
                if j == L:
                    break
                b = BT[j % 2]
                nc.vector.scalar_tensor_tensor(
                    b[:, :], osh, QS[:, 2 * HT + j:2 * HT + j + 1],
                    ES[:, eb:eb + HT], MULT, ADD)
                nc.vector.tensor_tensor_scan(
                    OS[:, ob + PW + 1:ob + PW + 1 + HT], b[:, 0:HT],
                    QS[:, HDR + j * HT:HDR + (j + 1) * HT], 0.0, ADD, MULT)
                if j == L - 1:
                    # all O-finals ready; export them under the last E-scan
                    nc.vector.tensor_copy(EX[:, L + 1:S], OS[:, HT::PW])
                    nc.sync.dma_start(afin[:, L + 1:132], EX[:, L + 1:132])
            nc.vector.memset(EX[:, 0:1], 0.0)
            nc.vector.tensor_copy(EX[:, 1:L + 1], ES[:, PW + HT::PW])
            nc.sync.dma_start(afin[:, 0:L + 1], EX[:, 0:L + 1])
    nc.compile()
    return nc


def _host_prep(y_true, y_pred):
    yt = np.asarray(y_true)
    yp = np.asarray(y_pred, dtype=np.float32)
    pB = yp[:, :, BLANK].astype(np.float64) + EPS            # [B, T]
    pl = (np.take_along_axis(yp, yt[:, None, :].astype(np.int64), axis=2)
          .astype(np.float64) + EPS)                          # [B, T, L]

    # fwd chain (t < HT) and bwd chain (reversed time + labels)
    q_f = pl[:, :HT, :] / pB[:, :HT, None]
    q_b = pl[:, :HT - 1:-1, ::-1] / pB[:, :HT - 1:-1, None]
    K_f = np.zeros((B, L))
    K_f[:, 1:] = (yt[:, 1:] != yt[:, :-1]).astype(np.float64)
    K_b = np.zeros((B, L))
    K_b[:, 1:] = (yt[:, ::-1][:, 1:] != yt[:, ::-1][:, :-1]).astype(np.float64)

    def chain_params(q):
        lnq = np.log(q).mean(axis=(1, 2))
        slope = CF0 * lnq + CF1
        gam = np.exp(-slope)
        r = gam ** (2 * TILT_EVERY)
        # damping from tilted 2-state mean-field surrogate
        e = np.ones((B,)); o = np.zeros((B,))
        g = np.empty((B, HT))
        qb = q.mean(axis=2)
        for t in range(HT):
            e2 = e + gam * o
            o2 = qb[:, t] * (o + gam * e + gam * gam * o)
            z2 = e2 + o2
            g[:, t] = z2 / (e + o)
            e, o = e2 / z2, o2 / z2
        delta = np.exp(-22.0 / 256.0) / g
        return r, delta

    r_f, d_f = chain_params(q_f)
    r_b, d_b = chain_params(q_b)

    def pack(q, K, r, delta):
        # qs rows: [dl | E0 series (shifted cumprod) | K' | q*delta series]
        n = q.shape[0]
        qt = (q * delta[:, :, None]).transpose(0, 2, 1)       # [n, L, HT]
        kc = K.copy()
        for j in range(TILT_EVERY, L, TILT_EVERY):
            kc[:, j] *= r
        ecp = np.ones((n, HT))
        ecp[:, 1:] = np.cumprod(delta[:, :HT - 1], axis=1)
        qs = np.concatenate(
            [delta, ecp, kc, qt.reshape(n, L * HT)], axis=1).astype(bf16)
        return qs, r.astype(np.float32)

    qs_f, gr_f = pack(q_f, K_f, r_f, d_f)
    qs_b, gr_b = pack(q_b, K_b, r_b, d_b)

    in_maps = []
    for ci in range(NCORES):
        ex = slice(ci * BS, (ci + 1) * BS)
        in_maps.append({
            "qs": np.concatenate([qs_f[ex], qs_b[ex]], axis=0),
            "gr": np.concatenate([gr_f[ex], gr_b[ex]], axis=0)[:, None],
        })
    aux = (pB, r_f, r_b, d_f, d_b, yt)
    return in_maps, aux


def _host_combine(afin, aux):
    pB, r_f, r_b, d_f, d_b, yt = aux
    af_s = afin[:, :BS, :].reshape(B, 132).astype(np.float64)
    ab_s = afin[:, BS:, :].reshape(B, 132).astype(np.float64)
    af_s[:, 0] = np.exp(np.log(d_f).sum(axis=1))
    ab_s[:, 0] = np.exp(np.log(d_b).sum(axis=1))

    # un-tilt ledger: pair j carries floor(j / TILT_EVERY) factors of r
    nt = np.floor_divide(np.arange(L + 1), TILT_EVERY)
    af = np.zeros((B, S)); ab = np.zeros((B, S))
    af[:, 0::2] = af_s[:, 0:L + 1] * r_f[:, None] ** (-nt[None, :])
    af[:, 1::2] = af_s[:, L + 1:S] * r_f[:, None] ** (-nt[None, :L])
    ab[:, 0::2] = ab_s[:, 0:L + 1] * r_b[:, None] ** (-nt[None, :])
    ab[:, 1::2] = ab_s[:, L + 1:S] * r_b[:, None] ** (-nt[None, :L])

    ext = np.full((B, S), BLANK, np.int64)
    ext[:, 1::2] = yt
    cs = np.zeros((B, S))
    cs[:, 2:] = ((ext[:, 2:] != BLANK)
                 & (ext[:, 2:] != ext[:, :-2])).astype(np.float64)
    zg = np.zeros((B, S + 2))
    zg[:, 2:] = af
    z = zg[:, 2:] + zg[:, 1:-1] + cs * zg[:, 0:-2]
    dot = (z * ab[:, ::-1]).sum(axis=1)

    lnF = np.log(pB).sum(axis=1)
    lnD = np.log(d_f).sum(axis=1) + np.log(d_b).sum(axis=1)
    ll = np.log(np.maximum(dot, 1e-300)) + lnF - lnD
    return (-ll[:, None]).astype(np.float32)


def kernel(y_true, y_pred):
    in_maps, aux = _host_prep(y_true, y_pred)
    if "nc" not in _CACHE:
        _CACHE["nc"] = _build_program()
    nc = _CACHE["nc"]
    res = run_bass_kernel_spmd(nc, in_maps, core_ids=list(range(NCORES)))
    afin = np.stack([np.asarray(res.results[i]["afin"], dtype=np.float32)
                     for i in range(NCORES)])
    return _host_combine(afin, aux)


# revision 19
# speedup vs baseline: 1.0937x; 1.0748x over previous
"""CTC loss (keras ctc_batch_cost semantics) on 8 Trainium2 NeuronCores.

Parity-normalized scan formulation. The CTC extended-state DP alternates
blank (even) and label (odd) states. All even states emit the same blank
probability pB_t, so normalizing the whole state vector by the running
blank product turns every even-state update into a pure shift-add and
every odd-state update into an affine recurrence in the emission RATIO
q_t = pl_t/pB_t. Each state-pair then reduces to three DVE instructions
over the full 256-step time axis:

    scanE:  E_t = delta_t * (E_{t-1} + O[j-1]_{t-1})     (cumsum-scan)
    stt:    b_t = K'_j * O[j-1]_{t-1} + E[j]_{t-1}       (fused mul-add)
    scanO:  O_t = qt[j]_t * (O_{t-1} + b_t)              (affine scan)

where qt = q * delta folds a per-(example,t) damping series delta chosen
from a mean-field surrogate so stored magnitudes stay O(1), and a
per-example tilt r applied every TILT_EVERY pairs (one tensor_scalar plus
host-folded K' = K*r) flattens the exponential state profile so bf16
storage holds the junction products. Forward (t<256) and reverse
(t>=256, states reversed) chains for 64 examples pack the 128 partitions
of each core; the final columns are gathered with two strided copies and
combined on the host in f64 with exact log-corrections for the blank
product, damping, and tilt ledgers.
"""
import numpy as np
import ml_dtypes

import concourse.bass as bass
import concourse.bacc as bacc
import concourse.mybir as mybir
from concourse import tile
from concourse.bass_utils import run_bass_kernel_spmd

B, T, C, L = 512, 512, 128, 64
S = 2 * L + 1
NCORES = 8
BS = B // NCORES        # 64 examples per core
HT = T // 2             # 256 timesteps per chain
BLANK = C - 1
EPS = 1e-7
TILT_EVERY = 16
PW = HT + 1             # per-pair series stride (1 guard col + 256)
QCHUNKS = (2, 6, 8, 8, 8, 8, 8, 8, 8)   # QS DMA chunk sizes in pairs
CF0, CF1 = 1.5689666, 2.17334313   # junction profile slope vs mean ln q

F32 = mybir.dt.float32
BF16 = mybir.dt.bfloat16
ADD = mybir.AluOpType.add
MULT = mybir.AluOpType.mult
bf16 = ml_dtypes.bfloat16

_CACHE = {}


HDR = 2 * HT + L        # header cols: dl series + E[0] series + kc columns


def _build_program():
    nc = bacc.Bacc("TRN2", target_bir_lowering=False, debug=False)
    qs = nc.dram_tensor("qs", [128, HDR + L * HT], BF16, kind="ExternalInput")
    gr = nc.dram_tensor("gr", [128, 1], F32, kind="ExternalInput")
    afin = nc.dram_tensor("afin", [128, 132], BF16, kind="ExternalOutput")

    with tile.TileContext(nc) as tc:
        with tc.tile_pool(name="static", bufs=1) as sp:
            ES = sp.tile([128, (L + 1) * PW], BF16)
            OS = sp.tile([128, L * PW], BF16)
            QS = sp.tile([128, HDR + L * HT], BF16)
            GR = sp.tile([128, 1], F32)
            GT = [sp.tile([128, HT], BF16, name=f"gt{i}") for i in range(2)]
            BT = [sp.tile([128, HT], BF16, name=f"bt{i}") for i in range(2)]
            EX = sp.tile([128, 132], BF16)
            # header + first pairs land in chunk 0; gr is not needed until
            # the first tilt hop (pair 8), so its DMA rides later
            pos = 0
            for i, npair in enumerate(QCHUNKS):
                a = 0 if i == 0 else HDR + pos * HT
                b = HDR + (pos + npair) * HT
                nc.sync.dma_start(QS[:, a:b], qs[:, a:b])
                pos += npair
                if i == 1:
                    nc.sync.dma_start(GR[:, :], gr[:, :])
            # guard zeros: each truncated pair-j E scan leaves one col
            # (region col j-1, absolute 258j-1) read by the stt; pair 1
            # reads OS pair-0 guard col 0
            nc.vector.memset(ES[:, PW:258 * L:PW + 1], 0.0)
            nc.vector.memset(OS[:, 0:1], 0.0)
            nc.vector.memset(EX[:, 129:132], 0.0)

            # pair 0: E[0] is the shipped delta-cumprod and K_0 = 0, so
            # b_0 is just its shift -- one scan total
            nc.vector.tensor_tensor_scan(
                OS[:, 1:1 + HT], QS[:, HT:2 * HT],
                QS[:, HDR:HDR + HT], 0.0, ADD, MULT)
            for j in range(1, L + 1):
                # wavefront: pair-j series are exactly zero for t < j-1, so
                # every op truncates to cols [st, 256)
                st = j - 1
                w = HT - st
                ob = (j - 1) * PW
                osh = OS[:, ob + st:ob + HT]
                tilted = (j % TILT_EVERY == 0)
                if tilted:
                    g = GT[(j // TILT_EVERY) % 2]
                    nc.vector.tensor_scalar_mul(g[:, 0:w], osh, GR[:, 0:1])
                    d0e = g[:, 0:w]
                else:
                    d0e = osh
                eb = j * PW
                nc.vector.tensor_tensor_scan(
                    ES[:, eb + 1 + st:eb + 1 + HT], d0e,
                    QS[:, st:HT], 0.0, ADD, MULT)
                if j == L:
                    break
                b = BT[j % 2]
                nc.vector.scalar_tensor_tensor(
                    b[:, 0:w], osh, QS[:, 2 * HT + j:2 * HT + j + 1],
                    ES[:, eb + st:eb + HT], MULT, ADD)
                nc.vector.tensor_tensor_scan(
                    OS[:, ob + PW + 1 + st:ob + PW + 1 + HT], b[:, 0:w],
                    QS[:, HDR + j * HT + st:HDR + (j + 1) * HT], 0.0, ADD, MULT)
                if j == L - 1:
                    # all O-finals ready; export them under the last E-scan
                    nc.vector.tensor_copy(EX[:, L + 1:S], OS[:, HT::PW])
                    nc.sync.dma_start(afin[:, L + 1:132], EX[:, L + 1:132])
            nc.vector.memset(EX[:, 0:1], 0.0)
            nc.vector.tensor_copy(EX[:, 1:L + 1], ES[:, PW + HT::PW])
            nc.sync.dma_start(afin[:, 0:L + 1], EX[:, 0:L + 1])
    nc.compile()
    return nc


def _host_prep(y_true, y_pred):
    yt = np.asarray(y_true)
    yp = np.asarray(y_pred, dtype=np.float32)
    pB = yp[:, :, BLANK].astype(np.float64) + EPS            # [B, T]
    pl = (np.take_along_axis(yp, yt[:, None, :].astype(np.int64), axis=2)
          .astype(np.float64) + EPS)                          # [B, T, L]

    # fwd chain (t < HT) and bwd chain (reversed time + labels)
    q_f = pl[:, :HT, :] / pB[:, :HT, None]
    q_b = pl[:, :HT - 1:-1, ::-1] / pB[:, :HT - 1:-1, None]
    K_f = np.zeros((B, L))
    K_f[:, 1:] = (yt[:, 1:] != yt[:, :-1]).astype(np.float64)
    K_b = np.zeros((B, L))
    K_b[:, 1:] = (yt[:, ::-1][:, 1:] != yt[:, ::-1][:, :-1]).astype(np.float64)

    def chain_params(q):
        lnq = np.log(q).mean(axis=(1, 2))
        slope = CF0 * lnq + CF1
        gam = np.exp(-slope)
        r = gam ** (2 * TILT_EVERY)
        # damping from tilted 2-state mean-field surrogate
        e = np.ones((B,)); o = np.zeros((B,))
        g = np.empty((B, HT))
        qb = q.mean(axis=2)
        for t in range(HT):
            e2 = e + gam * o
            o2 = qb[:, t] * (o + gam * e + gam * gam * o)
            z2 = e2 + o2
            g[:, t] = z2 / (e + o)
            e, o = e2 / z2, o2 / z2
        delta = np.exp(-22.0 / 256.0) / g
        return r, delta

    r_f, d_f = chain_params(q_f)
    r_b, d_b = chain_params(q_b)

    def pack(q, K, r, delta):
        # qs rows: [dl | E0 series (shifted cumprod) | K' | q*delta series]
        n = q.shape[0]
        qt = (q * delta[:, :, None]).transpose(0, 2, 1)       # [n, L, HT]
        kc = K.copy()
        for j in range(TILT_EVERY, L, TILT_EVERY):
            kc[:, j] *= r
        ecp = np.ones((n, HT))
        ecp[:, 1:] = np.cumprod(delta[:, :HT - 1], axis=1)
        qs = np.concatenate(
            [delta, ecp, kc, qt.reshape(n, L * HT)], axis=1).astype(bf16)
        return qs, r.astype(np.float32)

    qs_f, gr_f = pack(q_f, K_f, r_f, d_f)
    qs_b, gr_b = pack(q_b, K_b, r_b, d_b)

    in_maps = []
    for ci in range(NCORES):
        ex = slice(ci * BS, (ci + 1) * BS)
        in_maps.append({
            "qs": np.concatenate([qs_f[ex], qs_b[ex]], axis=0),
            "gr": np.concatenate([gr_f[ex], gr_b[ex]], axis=0)[:, None],
        })
    aux = (pB, r_f, r_b, d_f, d_b, yt)
    return in_maps, aux


def _host_combine(afin, aux):
    pB, r_f, r_b, d_f, d_b, yt = aux
    af_s = afin[:, :BS, :].reshape(B, 132).astype(np.float64)
    ab_s = afin[:, BS:, :].reshape(B, 132).astype(np.float64)
    af_s[:, 0] = np.exp(np.log(d_f).sum(axis=1))
    ab_s[:, 0] = np.exp(np.log(d_b).sum(axis=1))

    # un-tilt ledger: pair j carries floor(j / TILT_EVERY) factors of r
    nt = np.floor_divide(np.arange(L + 1), TILT_EVERY)
    af = np.zeros((B, S)); ab = np.zeros((B, S))
    af[:, 0::2] = af_s[:, 0:L + 1] * r_f[:, None] ** (-nt[None, :])
    af[:, 1::2] = af_s[:, L + 1:S] * r_f[:, None] ** (-nt[None, :L])
    ab[:, 0::2] = ab_s[:, 0:L + 1] * r_b[:, None] ** (-nt[None, :])
    ab[:, 1::2] = ab_s[:, L + 1:S] * r_b[:, None] ** (-nt[None, :L])

    ext = np.full((B, S), BLANK, np.int64)
    ext[:, 1::2] = yt
    cs = np.zeros((B, S))
    cs[:, 2:] = ((ext[:, 2:] != BLANK)
                 & (ext[:, 2:] != ext[:, :-2])).astype(np.float64)
    zg = np.zeros((B, S + 2))
    zg[:, 2:] = af
    z = zg[:, 2:] + zg[:, 1:-1] + cs * zg[:, 0:-2]
    dot = (z * ab[:, ::-1]).sum(axis=1)

    lnF = np.log(pB).sum(axis=1)
    lnD = np.log(d_f).sum(axis=1) + np.log(d_b).sum(axis=1)
    ll = np.log(np.maximum(dot, 1e-300)) + lnF - lnD
    return (-ll[:, None]).astype(np.float32)


def kernel(y_true, y_pred):
    in_maps, aux = _host_prep(y_true, y_pred)
    if "nc" not in _CACHE:
        _CACHE["nc"] = _build_program()
    nc = _CACHE["nc"]
    res = run_bass_kernel_spmd(nc, in_maps, core_ids=list(range(NCORES)))
    afin = np.stack([np.asarray(res.results[i]["afin"], dtype=np.float32)
                     for i in range(NCORES)])
    return _host_combine(afin, aux)


# revision 20
# speedup vs baseline: 1.0964x; 1.0024x over previous
"""CTC loss (keras ctc_batch_cost semantics) on 8 Trainium2 NeuronCores.

Parity-normalized scan formulation. The CTC extended-state DP alternates
blank (even) and label (odd) states. All even states emit the same blank
probability pB_t, so normalizing the whole state vector by the running
blank product turns every even-state update into a pure shift-add and
every odd-state update into an affine recurrence in the emission RATIO
q_t = pl_t/pB_t. Each state-pair then reduces to three DVE instructions
over the full 256-step time axis:

    scanE:  E_t = delta_t * (E_{t-1} + O[j-1]_{t-1})     (cumsum-scan)
    stt:    b_t = K'_j * O[j-1]_{t-1} + E[j]_{t-1}       (fused mul-add)
    scanO:  O_t = qt[j]_t * (O_{t-1} + b_t)              (affine scan)

where qt = q * delta folds a per-(example,t) damping series delta chosen
from a mean-field surrogate so stored magnitudes stay O(1), and a
per-example tilt r applied every TILT_EVERY pairs (one tensor_scalar plus
host-folded K' = K*r) flattens the exponential state profile so bf16
storage holds the junction products. Forward (t<256) and reverse
(t>=256, states reversed) chains for 64 examples pack the 128 partitions
of each core; the final columns are gathered with two strided copies and
combined on the host in f64 with exact log-corrections for the blank
product, damping, and tilt ledgers.
"""
import numpy as np
import ml_dtypes

import concourse.bass as bass
import concourse.bacc as bacc
import concourse.mybir as mybir
from concourse import tile
from concourse.bass_utils import run_bass_kernel_spmd

B, T, C, L = 512, 512, 128, 64
S = 2 * L + 1
NCORES = 8
BS = B // NCORES        # 64 examples per core
HT = T // 2             # 256 timesteps per chain
BLANK = C - 1
EPS = 1e-7
TILT_EVERY = 16
PW = HT + 1             # per-pair series stride (1 guard col + 256)
QCHUNKS = (2, 6, 8, 8, 8, 8, 8, 8, 8)   # QS DMA chunk sizes in pairs
CF0, CF1 = 1.5689666, 2.17334313   # junction profile slope vs mean ln q

F32 = mybir.dt.float32
BF16 = mybir.dt.bfloat16
ADD = mybir.AluOpType.add
MULT = mybir.AluOpType.mult
bf16 = ml_dtypes.bfloat16

_CACHE = {}


HDR = 2 * HT + L        # header cols: dl series + E[0] series + kc columns


def _build_program():
    nc = bacc.Bacc("TRN2", target_bir_lowering=False, debug=False)
    qs = nc.dram_tensor("qs", [128, HDR + L * HT], BF16, kind="ExternalInput")
    gr = nc.dram_tensor("gr", [128, 1], F32, kind="ExternalInput")
    afin = nc.dram_tensor("afin", [128, 132], BF16, kind="ExternalOutput")

    with tile.TileContext(nc) as tc:
        with tc.tile_pool(name="static", bufs=1) as sp:
            ES = sp.tile([128, (L + 1) * PW], BF16)
            OS = sp.tile([128, L * PW], BF16)
            QS = sp.tile([128, HDR + L * HT], BF16)
            GR = sp.tile([128, 1], F32)
            GT = [sp.tile([128, HT], BF16, name=f"gt{i}") for i in range(2)]
            BT = [sp.tile([128, HT], BF16, name=f"bt{i}") for i in range(2)]
            EX = sp.tile([128, 132], BF16)
            # header + first pairs land in chunk 0; gr is not needed until
            # the first tilt hop (pair 8), so its DMA rides later
            pos = 0
            for i, npair in enumerate(QCHUNKS):
                a = 0 if i == 0 else HDR + pos * HT
                b = HDR + (pos + npair) * HT
                nc.sync.dma_start(QS[:, a:b], qs[:, a:b])
                pos += npair
                if i == 1:
                    nc.sync.dma_start(GR[:, :], gr[:, :])
            # guard zeros: each truncated pair-j E scan leaves one col
            # (region col j-1, absolute 258j-1) read by the stt; pair 1
            # reads OS pair-0 guard col 0
            nc.vector.memset(ES[:, PW + 1:258 * L + 2:PW + 1], 0.0)
            nc.vector.memset(EX[:, 129:132], 0.0)

            # pair 0: E[0] is the shipped delta-cumprod and K_0 = 0, so
            # b_0 is just its shift -- one scan total
            nc.vector.tensor_tensor_scan(
                OS[:, 1:1 + HT], QS[:, HT:2 * HT],
                QS[:, HDR:HDR + HT], 0.0, ADD, MULT)
            for j in range(1, L + 1):
                # wavefront: pair-j series are exactly zero for t < j, so
                # every op truncates to cols [st, 256)
                st = j
                w = HT - st
                ob = (j - 1) * PW
                osh = OS[:, ob + st:ob + HT]
                tilted = (j % TILT_EVERY == 0)
                if tilted:
                    g = GT[(j // TILT_EVERY) % 2]
                    nc.vector.tensor_scalar_mul(g[:, 0:w], osh, GR[:, 0:1])
                    d0e = g[:, 0:w]
                else:
                    d0e = osh
                eb = j * PW
                nc.vector.tensor_tensor_scan(
                    ES[:, eb + 1 + st:eb + 1 + HT], d0e,
                    QS[:, st:HT], 0.0, ADD, MULT)
                if j == L:
                    break
                b = BT[j % 2]
                nc.vector.scalar_tensor_tensor(
                    b[:, 0:w], osh, QS[:, 2 * HT + j:2 * HT + j + 1],
                    ES[:, eb + st:eb + HT], MULT, ADD)
                nc.vector.tensor_tensor_scan(
                    OS[:, ob + PW + 1 + st:ob + PW + 1 + HT], b[:, 0:w],
                    QS[:, HDR + j * HT + st:HDR + (j + 1) * HT], 0.0, ADD, MULT)
                if j == L - 1:
                    # all O-finals ready; export them under the last E-scan
                    nc.vector.tensor_copy(EX[:, L + 1:S], OS[:, HT::PW])
                    nc.sync.dma_start(afin[:, L + 1:132], EX[:, L + 1:132])
            nc.vector.memset(EX[:, 0:1], 0.0)
            nc.vector.tensor_copy(EX[:, 1:L + 1], ES[:, PW + HT::PW])
            nc.sync.dma_start(afin[:, 0:L + 1], EX[:, 0:L + 1])
    nc.compile()
    return nc


def _host_prep(y_true, y_pred):
    yt = np.asarray(y_true)
    yp = np.asarray(y_pred, dtype=np.float32)
    pB = yp[:, :, BLANK].astype(np.float64) + EPS            # [B, T]
    pl = (np.take_along_axis(yp, yt[:, None, :].astype(np.int64), axis=2)
          .astype(np.float64) + EPS)                          # [B, T, L]

    # fwd chain (t < HT) and bwd chain (reversed time + labels)
    q_f = pl[:, :HT, :] / pB[:, :HT, None]
    q_b = pl[:, :HT - 1:-1, ::-1] / pB[:, :HT - 1:-1, None]
    K_f = np.zeros((B, L))
    K_f[:, 1:] = (yt[:, 1:] != yt[:, :-1]).astype(np.float64)
    K_b = np.zeros((B, L))
    K_b[:, 1:] = (yt[:, ::-1][:, 1:] != yt[:, ::-1][:, :-1]).astype(np.float64)

    def chain_params(q):
        lnq = np.log(q).mean(axis=(1, 2))
        slope = CF0 * lnq + CF1
        gam = np.exp(-slope)
        r = gam ** (2 * TILT_EVERY)
        # damping from tilted 2-state mean-field surrogate
        e = np.ones((B,)); o = np.zeros((B,))
        g = np.empty((B, HT))
        qb = q.mean(axis=2)
        for t in range(HT):
            e2 = e + gam * o
            o2 = qb[:, t] * (o + gam * e + gam * gam * o)
            z2 = e2 + o2
            g[:, t] = z2 / (e + o)
            e, o = e2 / z2, o2 / z2
        delta = np.exp(-22.0 / 256.0) / g
        return r, delta

    r_f, d_f = chain_params(q_f)
    r_b, d_b = chain_params(q_b)

    def pack(q, K, r, delta):
        # qs rows: [dl | E0 series (shifted cumprod) | K' | q*delta series]
        n = q.shape[0]
        qt = (q * delta[:, :, None]).transpose(0, 2, 1)       # [n, L, HT]
        kc = K.copy()
        for j in range(TILT_EVERY, L, TILT_EVERY):
            kc[:, j] *= r
        ecp = np.ones((n, HT))
        ecp[:, 1:] = np.cumprod(delta[:, :HT - 1], axis=1)
        qs = np.concatenate(
            [delta, ecp, kc, qt.reshape(n, L * HT)], axis=1).astype(bf16)
        return qs, r.astype(np.float32)

    qs_f, gr_f = pack(q_f, K_f, r_f, d_f)
    qs_b, gr_b = pack(q_b, K_b, r_b, d_b)

    in_maps = []
    for ci in range(NCORES):
        ex = slice(ci * BS, (ci + 1) * BS)
        in_maps.append({
            "qs": np.concatenate([qs_f[ex], qs_b[ex]], axis=0),
            "gr": np.concatenate([gr_f[ex], gr_b[ex]], axis=0)[:, None],
        })
    aux = (pB, r_f, r_b, d_f, d_b, yt)
    return in_maps, aux


def _host_combine(afin, aux):
    pB, r_f, r_b, d_f, d_b, yt = aux
    af_s = afin[:, :BS, :].reshape(B, 132).astype(np.float64)
    ab_s = afin[:, BS:, :].reshape(B, 132).astype(np.float64)
    af_s[:, 0] = np.exp(np.log(d_f).sum(axis=1))
    ab_s[:, 0] = np.exp(np.log(d_b).sum(axis=1))

    # un-tilt ledger: pair j carries floor(j / TILT_EVERY) factors of r
    nt = np.floor_divide(np.arange(L + 1), TILT_EVERY)
    af = np.zeros((B, S)); ab = np.zeros((B, S))
    af[:, 0::2] = af_s[:, 0:L + 1] * r_f[:, None] ** (-nt[None, :])
    af[:, 1::2] = af_s[:, L + 1:S] * r_f[:, None] ** (-nt[None, :L])
    ab[:, 0::2] = ab_s[:, 0:L + 1] * r_b[:, None] ** (-nt[None, :])
    ab[:, 1::2] = ab_s[:, L + 1:S] * r_b[:, None] ** (-nt[None, :L])

    ext = np.full((B, S), BLANK, np.int64)
    ext[:, 1::2] = yt
    cs = np.zeros((B, S))
    cs[:, 2:] = ((ext[:, 2:] != BLANK)
                 & (ext[:, 2:] != ext[:, :-2])).astype(np.float64)
    zg = np.zeros((B, S + 2))
    zg[:, 2:] = af
    z = zg[:, 2:] + zg[:, 1:-1] + cs * zg[:, 0:-2]
    dot = (z * ab[:, ::-1]).sum(axis=1)

    lnF = np.log(pB).sum(axis=1)
    lnD = np.log(d_f).sum(axis=1) + np.log(d_b).sum(axis=1)
    ll = np.log(np.maximum(dot, 1e-300)) + lnF - lnD
    return (-ll[:, None]).astype(np.float32)


def kernel(y_true, y_pred):
    in_maps, aux = _host_prep(y_true, y_pred)
    if "nc" not in _CACHE:
        _CACHE["nc"] = _build_program()
    nc = _CACHE["nc"]
    res = run_bass_kernel_spmd(nc, in_maps, core_ids=list(range(NCORES)))
    afin = np.stack([np.asarray(res.results[i]["afin"], dtype=np.float32)
                     for i in range(NCORES)])
    return _host_combine(afin, aux)
